# revision 61
# baseline (speedup 1.0000x reference)
"""Causal multi-head attention (B=2, S=2048, D=1024, H=16) on 8 TRN2 NeuronCores.

Sharding: core c handles batch b=c//4 and the 4 heads [4*(c%4), 4*(c%4)+4).
Each core computes its heads' Q/K/V projections, causal attention, and a
column-shard of the output projection; the host sums the 4 partials per batch
and adds bo_eff = bo + Wo@bv (v-bias folded out of the device kernel).

v3 design (v2 + tail/startup/pipeline fixes; ~165us, was ~172us):
  - everything in bf16 on the PE (fp8 DoubleRow was tried and measured SLOWER
    per unit work on this toolchain: 268ns vs 213ns per 512-col matmul)
  - block-causal at 128-k granularity on the diagonal 512x512 region of each
    q-block; causal mask as a 0/1 multiply on probs (DVE) for the triangular
    128x128 sub-tile of each diagonal k-tile
  - AV stationary carries a 65th ones-column -> row 64 of wv psum is the
    softmax denominator
  - software-pipelined emission: scores(kt) -> exp(kt) -> AV(kt-1), with the
    next block's projections and the previous block's O-projection injected
    as PE filler between attention steps
  - v3 fixes: the last block's p=1 normalize keeps z halves at partitions
    0..63 (no SBUF->SBUF partition-shift DMA on the critical path; the tail
    O-proj contracts them via a 64-row copy of Wo), reciprocal broadcast via
    PE outer product, fillers dumped before the last normalize to keep the PE
    p-state warm, tail psums alternate psG/psW/psS banks for 4-deep
    pipelining, startup x/wqk DMAs split per dk-chunk over 3 queues, tail
    outT writes in 512-col (1KB-line) chunks with the final transfer split
    across two queues
"""

import numpy as np

B, S, D, H = 2, 2048, 1024, 16
HD = D // H  # 64
NCORES = 8
P = 128
SB = 512          # s/q block size
NSB = S // SB     # 4
NDK = D // P      # 8

_CACHE = {}


def _build_nc():
    import concourse.bacc as bacc
    import concourse.mybir as mybir
    import concourse.tile as tile

    F32 = mybir.dt.float32
    BF16 = mybir.dt.bfloat16
    EXP = mybir.ActivationFunctionType.Exp
    MULT = mybir.AluOpType.mult

    nc = bacc.Bacc(None)
    xT = nc.declare_dram_parameter("xT", [D, S], BF16, isOutput=False)
    wqk = nc.declare_dram_parameter("wqkT", [D, 512], BF16, isOutput=False)
    wv = nc.declare_dram_parameter("wvT", [D, 256], BF16, isOutput=False)
    wo = nc.declare_dram_parameter("woT", [256, D], BF16, isOutput=False)
    woh = nc.declare_dram_parameter("woh", [64, 2 * D], BF16, isOutput=False)
    bqk = nc.declare_dram_parameter("bqk", [P, 4], F32, isOutput=False)
    tri = nc.declare_dram_parameter("tri", [P, P], BF16, isOutput=False)
    outT = nc.declare_dram_parameter("outT", [D, S], BF16, isOutput=True)

    with tile.TileContext(nc) as tc:
        with (
            tc.tile_pool(name="w", bufs=1) as wp,
            tc.tile_pool(name="x", bufs=2) as xp,
            tc.tile_pool(name="qk", bufs=1) as qkp,
            tc.tile_pool(name="pb", bufs=4) as pbp,
            tc.tile_pool(name="sm", bufs=2) as smp,
            tc.tile_pool(name="o", bufs=4) as op_,
            tc.tile_pool(name="psS", bufs=2, space="PSUM") as psS,   # sc: 2x2 banks
            tc.tile_pool(name="psW", bufs=1, space="PSUM") as psW,   # wv0+wv1: 2 banks
            tc.tile_pool(name="psG", bufs=2, space="PSUM") as psG,   # proj/oproj: 2 banks
        ):
            # ---- weights / constants (DMA order = need order, 3 queues;
            # block-0 x and wqk split per dk-chunk so the first matmul only
            # waits on 256KB) ----
            xT_r = xT[:].rearrange("(dk p) s -> p dk s", p=P)
            wqk_r = wqk[:].rearrange("(dk p) m -> p dk m", p=P)
            wqk_t = [wp.tile([P, 512], BF16, name=f"wqk{k}") for k in range(8)]
            x0 = [xp.tile([P, SB], BF16, tag=f"x0k{k}", name=f"x0k{k}")
                  for k in range(8)]
            dq = (nc.sync, nc.scalar, nc.gpsimd)
            for k in range(8):
                dq[(2 * k) % 3].dma_start(wqk_t[k][:], wqk_r[:, k, :])
                dq[(2 * k + 1) % 3].dma_start(x0[k][:], xT_r[:, k, 0:SB])
            bqk_sb = wp.tile([P, 4], F32)
            nc.scalar.dma_start(bqk_sb[:], bqk[:])
            tri_sb = wp.tile([P, P], BF16)
            nc.gpsimd.dma_start(tri_sb[:], tri[:])
            # gate the non-critical weight loads behind the last critical x
            # chunk so they don't steal HBM bandwidth from the startup path
            gate = wp.tile([1, 8], BF16)
            nc.sync.dma_start(gate[:], x0[7][0:1, 0:8])
            wv_sb = wp.tile([P, NDK, 256], BF16)
            nc.sync.dma_start(wv_sb[:], wv[:].rearrange("(dk p) m -> p dk m", p=P))
            wo_sb = wp.tile([P, 2, D], BF16)
            woh_sb = wp.tile([HD, 2, D], BF16)
            ones_b = wp.tile([1, HD], BF16)
            nc.vector.memset(ones_b[:], 1.0)

            # ---- persistent activations (split per block: no false deps) ----
            qT = [[qkp.tile([P, SB], BF16, tag=f"qT{p}b{b}", name=f"qT{p}b{b}")
                   for b in range(NSB)] for p in range(2)]
            kT = [[qkp.tile([P, SB], BF16, tag=f"kT{p}b{b}", name=f"kT{p}b{b}")
                   for b in range(NSB)] for p in range(2)]
            vt = [qkp.tile([P, 4, 4, HD + 1], BF16, tag=f"v{b}", name=f"v{b}") for b in range(NSB)]
            for b in range(NSB):
                nc.vector.memset(vt[b][:, :, :, HD:HD + 1], 1.0)
            zt = [qkp.tile([P, 2, SB], BF16, tag=f"zT{b}", name=f"zT{b}") for b in range(NSB)]
            # last block p=1 z halves stay at partitions 0..63 (no shift DMA)
            zh = [qkp.tile([HD, SB], BF16, tag=f"zh{h}", name=f"zh{h}") for h in range(2)]

            x_tiles = [x0]

            def xap(blk, dk, c0=0, c1=SB):
                """x chunk dk of block blk as a [P, c1-c0] AP."""
                xs = x_tiles[blk]
                if len(xs) == 8:
                    return xs[dk][:, c0:c1]
                return xs[dk // 2][:, dk % 2, c0:c1]

            # ---- emission helpers ----
            def proj_chunks(blk):
                """Closures emitting block blk's Q/K/V projections.

                Pair-0 q AND k first: the next block's p=0 scores need both,
                so their bias-adds must clear the DVE queue earliest."""
                out = []
                for pair in range(2):
                    for t in range(2):      # 0 = q, 1 = k
                        def qk_group(t=t, pair=pair, blk=blk):
                            ps = psG.tile([P, SB], F32, tag="g", name="g")
                            c0 = 256 * t + 128 * pair
                            for dk in range(NDK):
                                nc.tensor.matmul(
                                    ps[:], wqk_t[dk][:, c0:c0 + 128],
                                    xap(blk, dk),
                                    start=(dk == 0), stop=(dk == NDK - 1))
                            dst = (qT if t == 0 else kT)[pair][blk]
                            nc.vector.tensor_scalar_add(
                                dst[:], ps[:], bqk_sb[:, 2 * t + pair:2 * t + pair + 1])
                        out.append(qk_group)
                for half in range(2):        # two k-tiles of v per group
                    def v_group(half=half, blk=blk):
                        ps = psG.tile([P, 2, 256], F32, tag="g", name="gv")
                        for u in range(2):
                            c = 2 * half + u
                            for dk in range(NDK):
                                nc.tensor.matmul(
                                    ps[:, u, :],
                                    xap(blk, dk, c * P, (c + 1) * P),
                                    wv_sb[:, dk, :],
                                    start=(dk == 0), stop=(dk == NDK - 1))
                        nc.vector.tensor_copy(
                            vt[blk][:, 2 * half:2 * half + 2, :, 0:HD],
                            ps[:].rearrange("p two (h e) -> p two h e", h=4))
                    out.append(v_group)
                return out

            def oproj_chunks(j, alt=False):
                out = []
                for et in range(NDK):
                    def o_group(et=et, j=j, alt=alt):
                        if alt and et % 2 == 1:
                            # attention's sc banks are free once this emits
                            # (only released in the last-p dump)
                            pst = psS.tile([P, 2, SB], F32, tag="sc", name="gs")
                            ps = pst[:, 0, :]
                        else:
                            ps = psG.tile([P, SB], F32, tag="g", name="g")[:]
                        nc.tensor.matmul(ps, wo_sb[:, 0, et * P:(et + 1) * P],
                                         zt[j][:, 0, :], start=True, stop=False)
                        nc.tensor.matmul(ps, wo_sb[:, 1, et * P:(et + 1) * P],
                                         zt[j][:, 1, :], start=False, stop=True)
                        ot = op_.tile([P, SB], BF16, tag="ot", name="ot")
                        if alt and et >= 4:
                            nc.scalar.copy(ot[:], ps)
                        else:
                            nc.vector.tensor_copy(ot[:], ps)
                        nc.sync.dma_start(
                            outT[:][et * P:(et + 1) * P, j * SB:(j + 1) * SB], ot[:])
                    out.append(o_group)
                return out

            def x_prefetch(nblk):
                def go(nblk=nblk):
                    xn = [xp.tile([P, 2, SB], BF16, tag=f"xc{c}", name=f"x{nblk}c{c}")
                          for c in range(4)]
                    x_tiles.append(xn)
                    for c in range(4):
                        nc.sync.dma_start(
                            xn[c][:],
                            xT_r[:, 2 * c:2 * c + 2, nblk * SB:(nblk + 1) * SB])
                return go

            # ---- block 0 projections (nothing to interleave with yet) ----
            for ch in proj_chunks(0):
                ch()
            # wo/woh aren't needed until the first O-projection (~mid-kernel);
            # keep them off the sync queue so x-prefetch(1) isn't delayed, and
            # gate them behind gpsimd's last critical startup load
            gate2 = wp.tile([1, 8], BF16)
            nc.gpsimd.dma_start(gate2[:], wqk_t[7][0:1, 0:8])
            nc.gpsimd.dma_start(wo_sb[:], wo[:].rearrange("(k p) m -> p k m", p=P))
            nc.gpsimd.dma_start(woh_sb[:], woh[:].rearrange("p (k m) -> p k m", k=2))

            # ---- main loop: attention(j) + interleaved proj(j+1)/oproj(j-1) --
            for j in range(NSB):
                fillers = []
                if j + 1 < NSB:
                    fillers.append(x_prefetch(j + 1))
                    pj = proj_chunks(j + 1)
                else:
                    pj = []
                # defer O-projections so attn(2)/attn(3) have enough filler
                if j == 2:
                    oj = oproj_chunks(0)
                elif j == 3:
                    oj = oproj_chunks(1) + oproj_chunks(2, alt=True)
                else:
                    oj = []
                n = max(len(pj), len(oj))
                for i in range(n):
                    if i < len(pj):
                        fillers.append(pj[i])
                    if i < len(oj):
                        fillers.append(oj[i])

                total_slots = 2 * (4 * j + 4)
                slot = 0
                emitted = 0
                for p in range(2):
                    wv_t = [psW.tile([P, SB], F32, tag=f"wv{h}", name=f"wv{h}")
                            for h in range(2)]

                    def emit_av(item, p=p, j=j, wv_t=wv_t):
                        pr, koff, kti = item
                        first = (kti == 0)
                        last = (kti == 4 * j + 3)
                        for half in range(2):
                            nc.tensor.matmul(
                                wv_t[half][0:HD + 1, koff:SB],
                                vt[kti // 4][:, kti % 4, 2 * p + half, :],
                                pr[:, half, koff:SB],
                                start=first, stop=last, skip_group_check=True)

                    kts = list(range(4 * j)) + [("d", kk) for kk in range(4)]
                    prev = None
                    for i, kt in enumerate(kts):
                        if isinstance(kt, int):
                            koff = 0
                            kti = kt
                        else:
                            kk = kt[1]
                            koff = kk * P
                            kti = 4 * j + kk
                        sc = psS.tile([P, 2, SB], F32, tag="sc", name="sc")
                        for half in range(2):
                            base = HD * half
                            nc.tensor.matmul(
                                sc[:, half, koff:SB],
                                kT[p][kti // 4][base:base + HD,
                                                (kti % 4) * P:(kti % 4 + 1) * P],
                                qT[p][j][base:base + HD, koff:SB],
                                start=True, stop=True)
                        pr = pbp.tile([P, 2, SB], BF16, tag="pr", name="pr")
                        nc.scalar.activation(pr[:, :, koff:SB], sc[:, :, koff:SB], EXP)
                        if not isinstance(kt, int):
                            # 0/1 causal mask on the triangular 128x128 sub-tile
                            nc.vector.tensor_tensor(
                                pr[:, :, koff:koff + P], pr[:, :, koff:koff + P],
                                tri_sb[:, None, :].to_broadcast([P, 2, P]), MULT)
                        if prev is not None:
                            emit_av(prev)
                        prev = (pr, koff, kti)
                        slot += 1
                        # hold back fillers near each p's end so normalize's DVE
                        # ops aren't queued behind filler copies
                        if i < len(kts) - 2:
                            cap = len(fillers) - 8 if j == NSB - 1 else len(fillers)
                            want = min(cap,
                                       (slot * len(fillers) * 5)
                                       // (4 * max(1, total_slots - 4)))
                            while emitted < want:
                                fillers[emitted]()
                                emitted += 1
                    emit_av(prev)

                    # ---- normalize: denominators from psum row 64 ----
                    last_p = (j == NSB - 1 and p == 1)
                    if last_p:
                        # keep the PE warm through the normalize chain; the
                        # last 4 chunks (scalar copies) emit after normalize
                        while emitted < len(fillers) - 4:
                            fillers[emitted]()
                            emitted += 1
                    dns, wvss = [], []
                    for half in range(2):
                        # front-load the PSUM reads so the wv banks free up
                        # before the next p's first AV needs them
                        dn = smp.tile([1, SB], F32, tag=f"dn{half}",
                                      name=f"dn{half}")
                        wvs = smp.tile([HD, SB], F32, tag=f"wvs{half}",
                                       name=f"wvs{half}")
                        if last_p:
                            nc.scalar.copy(dn[:], wv_t[half][HD:HD + 1, :])
                            nc.scalar.copy(wvs[:], wv_t[half][0:HD, :])
                        else:
                            nc.vector.tensor_copy(dn[:], wv_t[half][HD:HD + 1, :])
                            nc.vector.tensor_copy(wvs[:], wv_t[half][0:HD, :])
                        dns.append(dn)
                        wvss.append(wvs)
                    for half in range(2):
                        dn, wvs = dns[half], wvss[half]
                        rr = smp.tile([1, SB], F32, tag="rr", name="rr")
                        nc.vector.reciprocal_approx_fast(rr[:], dn[:])
                        if last_p:
                            # short-latency path: PE outer-product broadcast,
                            # z halves stay at partitions 0..63
                            rrb = smp.tile([1, SB], BF16, tag="rrb", name="rrb")
                            nc.vector.tensor_copy(rrb[:], rr[:])
                            rbp = psG.tile([HD, SB], F32, tag="g", name="rbp")
                            nc.tensor.matmul(rbp[:], ones_b[:], rrb[:],
                                             start=True, stop=True)
                            nc.vector.tensor_tensor(
                                zh[half][:], wvs[:], rbp[:], MULT)
                        else:
                            rb = smp.tile([HD, SB], F32, tag="rb", name="rb")
                            nc.gpsimd.partition_broadcast(rb[:], rr[:])
                            if half == 0:
                                nc.vector.tensor_tensor(
                                    zt[j][0:HD, p, :], wvs[:], rb[:], MULT)
                            else:
                                zo = smp.tile([HD, SB], BF16, tag="zo", name="zo")
                                nc.vector.tensor_tensor(
                                    zo[:], wvs[:], rb[:], MULT)
                                nc.sync.dma_start(zt[j][HD:P, p, :], zo[:])

                while emitted < len(fillers):
                    fillers[emitted]()
                    emitted += 1

            # ---- tail: O-projection of the last block ----
            # zt[3][:,0,:] holds pair 0 (128-row contraction); pair 1 lives in
            # zh[0]/zh[1] at partitions 0..63, contracted via woh (64-row Wo).
            j = NSB - 1
            pss = {}
            for et in range(4):
                # pre-start the p0-half while the normalize chain runs;
                # zt[3][:,0,:] is ready well before zh[0]/zh[1]
                if et % 2 == 0:
                    ps = psG.tile([P, SB], F32, tag="g", name="gt")
                else:
                    ps = psW.tile([P, SB], F32, tag=f"wv{(et // 2) % 2}",
                                  name="gtw")
                pss[et] = ps
                nc.tensor.matmul(ps[:], wo_sb[:, 0, et * P:(et + 1) * P],
                                 zt[j][:, 0, :], start=True, stop=False)
            for et in range(NDK):
                if et < 4:
                    ps = pss[et]
                else:
                    if et % 2 == 0:
                        ps = psG.tile([P, SB], F32, tag="g", name="gt")
                    else:
                        ps = psW.tile([P, SB], F32, tag=f"wv{(et // 2) % 2}",
                                      name="gtw")
                    nc.tensor.matmul(ps[:], wo_sb[:, 0, et * P:(et + 1) * P],
                                     zt[j][:, 0, :], start=True, stop=False)
                nc.tensor.matmul(ps[:], woh_sb[:, 0, et * P:(et + 1) * P],
                                 zh[0][:], start=False, stop=False)
                nc.tensor.matmul(ps[:], woh_sb[:, 1, et * P:(et + 1) * P],
                                 zh[1][:], start=False, stop=True)
                ot = op_.tile([P, SB], BF16, tag="ott", name="ott")
                if et % 2 == 0:
                    nc.scalar.copy(ot[:], ps[:])
                else:
                    nc.vector.tensor_copy(ot[:], ps[:])
                if et < NDK - 1:
                    eng = nc.sync if et % 2 == 0 else nc.scalar
                    eng.dma_start(
                        outT[:][et * P:(et + 1) * P, j * SB:(j + 1) * SB], ot[:])
                else:
                    # split the final transfer across two queues to halve drain
                    nc.sync.dma_start(
                        outT[:][et * P:(et + 1) * P, j * SB:j * SB + 256],
                        ot[:, 0:256])
                    nc.scalar.dma_start(
                        outT[:][et * P:(et + 1) * P, j * SB + 256:(j + 1) * SB],
                        ot[:, 256:SB])

    nc.compile()
    return nc


def _host_inputs(inputs, Wq, bq, Wk, bk, Wv, bv, Wo, bo):
    """Build the 8 per-core input maps (bf16 weights/activations)."""
    import ml_dtypes
    bf16 = ml_dtypes.bfloat16
    scale = np.float32(1.0 / np.sqrt(HD))
    tri = np.ascontiguousarray(
        (np.arange(P)[:, None] <= np.arange(P)[None, :]).astype(bf16))
    xT_b = [np.ascontiguousarray(np.asarray(inputs[b], np.float32).T.astype(bf16))
            for b in range(B)]
    in_maps = []
    for c in range(NCORES):
        hg = c % 4
        hs = slice(4 * hg, 4 * hg + 4)
        WqT = np.asarray(Wq[hs], np.float32).transpose(2, 0, 1).reshape(D, 256) * scale
        WkT = np.asarray(Wk[hs], np.float32).transpose(2, 0, 1).reshape(D, 256)
        WvT = np.asarray(Wv[hs], np.float32).transpose(2, 0, 1).reshape(D, 256)
        wqkT = np.ascontiguousarray(
            np.concatenate([WqT, WkT], axis=1).astype(bf16))
        bq_c = np.asarray(bq[hs], np.float32).reshape(256) * scale
        bk_c = np.asarray(bk[hs], np.float32).reshape(256)
        bqk_c = np.stack([bq_c[0:128], bq_c[128:256], bk_c[0:128], bk_c[128:256]],
                         axis=1)
        woT = np.ascontiguousarray(
            np.asarray(Wo, np.float32)[:, 256 * hg:256 * (hg + 1)].T.astype(bf16))
        woh_c = np.ascontiguousarray(
            woT[128:256].reshape(2, 64, D).transpose(1, 0, 2).reshape(64, 2 * D))
        in_maps.append({
            "xT": xT_b[c // 4], "wqkT": wqkT,
            "wvT": np.ascontiguousarray(WvT.astype(bf16)), "woT": woT,
            "woh": woh_c,
            "bqk": np.ascontiguousarray(bqk_c), "tri": tri,
        })
    return in_maps


def _assemble(results, Wo, bv, bo):
    out = np.zeros((B, S, D), dtype=np.float32)
    for c in range(NCORES):
        out[c // 4] += results[c]["outT"].astype(np.float32).T
    bo_eff = (np.asarray(bo, np.float32)
              + np.asarray(Wo, np.float32) @ np.asarray(bv, np.float32).reshape(-1))
    out += bo_eff[None, None, :]
    return out


def kernel(inputs, Wq, bq, Wk, bk, Wv, bv, Wo, bo):
    from concourse.bass_utils import run_bass_kernel_spmd

    if "nc" not in _CACHE:
        _CACHE["nc"] = _build_nc()
    nc = _CACHE["nc"]
    in_maps = _host_inputs(inputs, Wq, bq, Wk, bk, Wv, bv, Wo, bo)
    res = run_bass_kernel_spmd(nc, in_maps, list(range(NCORES)))
    return _assemble(res.results, Wo, bv, bo)


# revision 63
# speedup vs baseline: 1.0329x; 1.0329x over previous
"""Causal multi-head attention (B=2, S=2048, D=1024, H=16) on 8 TRN2 NeuronCores.

Sharding: core c handles batch b=c//4 and the 4 heads [4*(c%4), 4*(c%4)+4).
Each core computes its heads' Q/K/V projections, causal attention, and a
column-shard of the output projection; the host sums the 4 partials per batch
and adds bo_eff = bo + Wo@bv (v-bias folded out of the device kernel).

v3 design (v2 + tail/startup/pipeline fixes; ~165us, was ~172us):
  - everything in bf16 on the PE (fp8 DoubleRow was tried and measured SLOWER
    per unit work on this toolchain: 268ns vs 213ns per 512-col matmul)
  - block-causal at 128-k granularity on the diagonal 512x512 region of each
    q-block; causal mask as a 0/1 multiply on probs (DVE) for the triangular
    128x128 sub-tile of each diagonal k-tile
  - AV stationary carries a 65th ones-column -> row 64 of wv psum is the
    softmax denominator
  - software-pipelined emission: scores(kt) -> exp(kt) -> AV(kt-1), with the
    next block's projections and the previous block's O-projection injected
    as PE filler between attention steps
  - v3 fixes: the last block's p=1 normalize keeps z halves at partitions
    0..63 (no SBUF->SBUF partition-shift DMA on the critical path; the tail
    O-proj contracts them via a 64-row copy of Wo), reciprocal broadcast via
    PE outer product, fillers dumped before the last normalize to keep the PE
    p-state warm, tail psums alternate psG/psW/psS banks for 4-deep
    pipelining, startup x/wqk DMAs split per dk-chunk over 3 queues, tail
    outT writes in 512-col (1KB-line) chunks with the final transfer split
    across two queues
"""

import numpy as np

B, S, D, H = 2, 2048, 1024, 16
HD = D // H  # 64
NCORES = 8
P = 128
SB = 512          # s/q block size
NSB = S // SB     # 4
NDK = D // P      # 8

_CACHE = {}


def _build_nc():
    import concourse.bacc as bacc
    import concourse.mybir as mybir
    import concourse.tile as tile

    F32 = mybir.dt.float32
    BF16 = mybir.dt.bfloat16
    EXP = mybir.ActivationFunctionType.Exp
    MULT = mybir.AluOpType.mult

    nc = bacc.Bacc(None)
    xT = nc.declare_dram_parameter("xT", [D, S], BF16, isOutput=False)
    wqk = nc.declare_dram_parameter("wqkT", [D, 512], BF16, isOutput=False)
    wv = nc.declare_dram_parameter("wvT", [D, 256], BF16, isOutput=False)
    wo = nc.declare_dram_parameter("woT", [256, D], BF16, isOutput=False)
    woh = nc.declare_dram_parameter("woh", [64, 2 * D], BF16, isOutput=False)
    bqk = nc.declare_dram_parameter("bqk", [P, 4], F32, isOutput=False)
    tri = nc.declare_dram_parameter("tri", [P, P], BF16, isOutput=False)
    outT = nc.declare_dram_parameter("outT", [D, S], BF16, isOutput=True)

    with tile.TileContext(nc) as tc:
        with (
            tc.tile_pool(name="w", bufs=1) as wp,
            tc.tile_pool(name="x", bufs=2) as xp,
            tc.tile_pool(name="qk", bufs=1) as qkp,
            tc.tile_pool(name="pb", bufs=4) as pbp,
            tc.tile_pool(name="sm", bufs=2) as smp,
            tc.tile_pool(name="o", bufs=4) as op_,
            tc.tile_pool(name="psS", bufs=2, space="PSUM") as psS,   # sc: 2x2 banks
            tc.tile_pool(name="psW", bufs=1, space="PSUM") as psW,   # wv0+wv1: 2 banks
            tc.tile_pool(name="psG", bufs=2, space="PSUM") as psG,   # proj/oproj: 2 banks
        ):
            # ---- weights / constants (DMA order = need order, 3 queues;
            # block-0 x and wqk split per dk-chunk so the first matmul only
            # waits on 256KB) ----
            xT_r = xT[:].rearrange("(dk p) s -> p dk s", p=P)
            wqk_r = wqk[:].rearrange("(dk p) m -> p dk m", p=P)
            wqk_t = [wp.tile([P, 512], BF16, name=f"wqk{k}") for k in range(8)]
            x0 = [xp.tile([P, SB], BF16, tag=f"x0k{k}", name=f"x0k{k}")
                  for k in range(8)]
            dq = (nc.sync, nc.scalar, nc.gpsimd)
            for k in range(8):
                dq[(2 * k) % 3].dma_start(wqk_t[k][:], wqk_r[:, k, :])
                dq[(2 * k + 1) % 3].dma_start(x0[k][:], xT_r[:, k, 0:SB])
            bqk_sb = wp.tile([P, 4], F32)
            nc.scalar.dma_start(bqk_sb[:], bqk[:])
            tri_sb = wp.tile([P, P], BF16)
            nc.gpsimd.dma_start(tri_sb[:], tri[:])
            # gate the non-critical weight loads behind the last critical x
            # chunk so they don't steal HBM bandwidth from the startup path
            gate = wp.tile([1, 8], BF16)
            nc.sync.dma_start(gate[:], x0[7][0:1, 0:8])
            wv_sb = wp.tile([P, NDK, 256], BF16)
            nc.sync.dma_start(wv_sb[:], wv[:].rearrange("(dk p) m -> p dk m", p=P))
            wo_sb = wp.tile([P, 2, D], BF16)
            woh_sb = wp.tile([HD, 2, D], BF16)
            ones_b = wp.tile([1, HD], BF16)
            nc.vector.memset(ones_b[:], 1.0)

            # ---- persistent activations (split per block: no false deps) ----
            qT = [[qkp.tile([P, SB], BF16, tag=f"qT{p}b{b}", name=f"qT{p}b{b}")
                   for b in range(NSB)] for p in range(2)]
            kT = [[qkp.tile([P, SB], BF16, tag=f"kT{p}b{b}", name=f"kT{p}b{b}")
                   for b in range(NSB)] for p in range(2)]
            vt = [qkp.tile([P, 4, 4, HD + 1], BF16, tag=f"v{b}", name=f"v{b}") for b in range(NSB)]
            for b in range(NSB):
                nc.vector.memset(vt[b][:, :, :, HD:HD + 1], 1.0)
            zt = [qkp.tile([P, 2, SB], BF16, tag=f"zT{b}", name=f"zT{b}") for b in range(NSB)]
            # last block p=1 z halves stay at partitions 0..63 (no shift DMA)
            zh = [qkp.tile([HD, SB], BF16, tag=f"zh{h}", name=f"zh{h}") for h in range(2)]

            x_tiles = [x0]

            def xap(blk, dk, c0=0, c1=SB):
                """x chunk dk of block blk as a [P, c1-c0] AP."""
                xs = x_tiles[blk]
                if len(xs) == 8:
                    return xs[dk][:, c0:c1]
                return xs[dk // 2][:, dk % 2, c0:c1]

            # ---- emission helpers ----
            def proj_chunks(blk):
                """Closures emitting block blk's Q/K/V projections."""
                out = []
                for t in range(2):          # 0 = q, 1 = k
                    for pair in range(2):
                        def qk_group(t=t, pair=pair, blk=blk):
                            ps = psG.tile([P, SB], F32, tag="g", name="g")
                            c0 = 256 * t + 128 * pair
                            for dk in range(NDK):
                                nc.tensor.matmul(
                                    ps[:], wqk_t[dk][:, c0:c0 + 128],
                                    xap(blk, dk),
                                    start=(dk == 0), stop=(dk == NDK - 1))
                            dst = (qT if t == 0 else kT)[pair][blk]
                            nc.vector.tensor_scalar_add(
                                dst[:], ps[:], bqk_sb[:, 2 * t + pair:2 * t + pair + 1])
                        out.append(qk_group)
                for half in range(2):        # two k-tiles of v per group
                    def v_group(half=half, blk=blk):
                        ps = psG.tile([P, 2, 256], F32, tag="g", name="gv")
                        for u in range(2):
                            c = 2 * half + u
                            for dk in range(NDK):
                                nc.tensor.matmul(
                                    ps[:, u, :],
                                    xap(blk, dk, c * P, (c + 1) * P),
                                    wv_sb[:, dk, :],
                                    start=(dk == 0), stop=(dk == NDK - 1))
                        nc.vector.tensor_copy(
                            vt[blk][:, 2 * half:2 * half + 2, :, 0:HD],
                            ps[:].rearrange("p two (h e) -> p two h e", h=4))
                    out.append(v_group)
                return out

            def oproj_chunks(j, alt=False):
                out = []
                for et in range(NDK):
                    def o_group(et=et, j=j, alt=alt):
                        if alt and et % 2 == 1:
                            # attention's sc banks are free once this emits
                            # (only released in the last-p dump)
                            pst = psS.tile([P, 2, SB], F32, tag="sc", name="gs")
                            ps = pst[:, 0, :]
                        else:
                            ps = psG.tile([P, SB], F32, tag="g", name="g")[:]
                        nc.tensor.matmul(ps, wo_sb[:, 0, et * P:(et + 1) * P],
                                         zt[j][:, 0, :], start=True, stop=False)
                        nc.tensor.matmul(ps, wo_sb[:, 1, et * P:(et + 1) * P],
                                         zt[j][:, 1, :], start=False, stop=True)
                        ot = op_.tile([P, SB], BF16, tag="ot", name="ot")
                        if alt and et >= 4:
                            nc.scalar.copy(ot[:], ps)
                        else:
                            nc.vector.tensor_copy(ot[:], ps)
                        nc.sync.dma_start(
                            outT[:][et * P:(et + 1) * P, j * SB:(j + 1) * SB], ot[:])
                    out.append(o_group)
                return out

            def x_prefetch(nblk):
                def go(nblk=nblk):
                    xn = [xp.tile([P, 2, SB], BF16, tag=f"xc{c}", name=f"x{nblk}c{c}")
                          for c in range(4)]
                    x_tiles.append(xn)
                    for c in range(4):
                        nc.sync.dma_start(
                            xn[c][:],
                            xT_r[:, 2 * c:2 * c + 2, nblk * SB:(nblk + 1) * SB])
                return go

            # ---- block 0 projections (nothing to interleave with yet) ----
            for ch in proj_chunks(0):
                ch()
            nc.sync.dma_start(wo_sb[:], wo[:].rearrange("(k p) m -> p k m", p=P))
            nc.sync.dma_start(woh_sb[:], woh[:].rearrange("p (k m) -> p k m", k=2))

            # ---- main loop: attention(j) + interleaved proj(j+1)/oproj(j-1) --
            for j in range(NSB):
                fillers = []
                if j + 1 < NSB:
                    fillers.append(x_prefetch(j + 1))
                    pj = proj_chunks(j + 1)
                else:
                    pj = []
                # defer O-projections so attn(2)/attn(3) have enough filler
                if j == 2:
                    oj = oproj_chunks(0)
                elif j == 3:
                    oj = oproj_chunks(1) + oproj_chunks(2, alt=True)
                else:
                    oj = []
                n = max(len(pj), len(oj))
                for i in range(n):
                    if i < len(pj):
                        fillers.append(pj[i])
                    if i < len(oj):
                        fillers.append(oj[i])

                total_slots = 2 * (4 * j + 4)
                slot = 0
                emitted = 0
                for p in range(2):
                    wv_t = [psW.tile([P, SB], F32, tag=f"wv{h}", name=f"wv{h}")
                            for h in range(2)]

                    def emit_av(item, p=p, j=j, wv_t=wv_t):
                        pr, koff, kti = item
                        first = (kti == 0)
                        last = (kti == 4 * j + 3)
                        for half in range(2):
                            nc.tensor.matmul(
                                wv_t[half][0:HD + 1, koff:SB],
                                vt[kti // 4][:, kti % 4, 2 * p + half, :],
                                pr[:, half, koff:SB],
                                start=first, stop=last, skip_group_check=True)

                    kts = list(range(4 * j)) + [("d", kk) for kk in range(4)]
                    prev = None
                    for i, kt in enumerate(kts):
                        if isinstance(kt, int):
                            koff = 0
                            kti = kt
                        else:
                            kk = kt[1]
                            koff = kk * P
                            kti = 4 * j + kk
                        sc = psS.tile([P, 2, SB], F32, tag="sc", name="sc")
                        for half in range(2):
                            base = HD * half
                            nc.tensor.matmul(
                                sc[:, half, koff:SB],
                                kT[p][kti // 4][base:base + HD,
                                                (kti % 4) * P:(kti % 4 + 1) * P],
                                qT[p][j][base:base + HD, koff:SB],
                                start=True, stop=True)
                        pr = pbp.tile([P, 2, SB], BF16, tag="pr", name="pr")
                        nc.scalar.activation(pr[:, :, koff:SB], sc[:, :, koff:SB], EXP)
                        if not isinstance(kt, int):
                            # 0/1 causal mask on the triangular 128x128 sub-tile
                            nc.vector.tensor_tensor(
                                pr[:, :, koff:koff + P], pr[:, :, koff:koff + P],
                                tri_sb[:, None, :].to_broadcast([P, 2, P]), MULT)
                        if prev is not None:
                            emit_av(prev)
                        prev = (pr, koff, kti)
                        slot += 1
                        # hold back fillers near each p's end so normalize's DVE
                        # ops aren't queued behind filler copies
                        if i < len(kts) - 2:
                            cap = len(fillers) - 8 if j == NSB - 1 else len(fillers)
                            want = min(cap,
                                       (slot * len(fillers) * 5)
                                       // (4 * max(1, total_slots - 4)))
                            while emitted < want:
                                fillers[emitted]()
                                emitted += 1
                    emit_av(prev)

                    # ---- normalize: denominators from psum row 64 ----
                    last_p = (j == NSB - 1 and p == 1)
                    if last_p:
                        # keep the PE warm through the normalize chain; the
                        # last 4 chunks (scalar copies) emit after normalize
                        while emitted < len(fillers) - 4:
                            fillers[emitted]()
                            emitted += 1
                    dns, wvss = [], []
                    for half in range(2):
                        # front-load the PSUM reads so the wv banks free up
                        # before the next p's first AV needs them
                        dn = smp.tile([1, SB], F32, tag=f"dn{half}",
                                      name=f"dn{half}")
                        wvs = smp.tile([HD, SB], F32, tag=f"wvs{half}",
                                       name=f"wvs{half}")
                        if last_p:
                            nc.scalar.copy(dn[:], wv_t[half][HD:HD + 1, :])
                            nc.scalar.copy(wvs[:], wv_t[half][0:HD, :])
                        else:
                            nc.vector.tensor_copy(dn[:], wv_t[half][HD:HD + 1, :])
                            nc.vector.tensor_copy(wvs[:], wv_t[half][0:HD, :])
                        dns.append(dn)
                        wvss.append(wvs)
                    for half in range(2):
                        dn, wvs = dns[half], wvss[half]
                        rr = smp.tile([1, SB], F32, tag="rr", name="rr")
                        nc.vector.reciprocal_approx_fast(rr[:], dn[:])
                        if last_p:
                            # short-latency path: PE outer-product broadcast,
                            # z halves stay at partitions 0..63
                            rrb = smp.tile([1, SB], BF16, tag="rrb", name="rrb")
                            nc.vector.tensor_copy(rrb[:], rr[:])
                            rbp = psG.tile([HD, SB], F32, tag="g", name="rbp")
                            nc.tensor.matmul(rbp[:], ones_b[:], rrb[:],
                                             start=True, stop=True)
                            nc.vector.tensor_tensor(
                                zh[half][:], wvs[:], rbp[:], MULT)
                        else:
                            rb = smp.tile([HD, SB], F32, tag="rb", name="rb")
                            nc.gpsimd.partition_broadcast(rb[:], rr[:])
                            if half == 0:
                                nc.vector.tensor_tensor(
                                    zt[j][0:HD, p, :], wvs[:], rb[:], MULT)
                            else:
                                zo = smp.tile([HD, SB], BF16, tag="zo", name="zo")
                                nc.vector.tensor_tensor(
                                    zo[:], wvs[:], rb[:], MULT)
                                nc.sync.dma_start(zt[j][HD:P, p, :], zo[:])

                while emitted < len(fillers):
                    fillers[emitted]()
                    emitted += 1

            # ---- tail: O-projection of the last block ----
            # zt[3][:,0,:] holds pair 0 (128-row contraction); pair 1 lives in
            # zh[0]/zh[1] at partitions 0..63, contracted via woh (64-row Wo).
            j = NSB - 1
            pss = {}
            for et in range(4):
                # pre-start the p0-half while the normalize chain runs;
                # zt[3][:,0,:] is ready well before zh[0]/zh[1]
                if et % 2 == 0:
                    ps = psG.tile([P, SB], F32, tag="g", name="gt")
                else:
                    ps = psW.tile([P, SB], F32, tag=f"wv{(et // 2) % 2}",
                                  name="gtw")
                pss[et] = ps
                nc.tensor.matmul(ps[:], wo_sb[:, 0, et * P:(et + 1) * P],
                                 zt[j][:, 0, :], start=True, stop=False)
            for et in range(NDK):
                if et < 4:
                    ps = pss[et]
                else:
                    if et % 2 == 0:
                        ps = psG.tile([P, SB], F32, tag="g", name="gt")
                    else:
                        ps = psW.tile([P, SB], F32, tag=f"wv{(et // 2) % 2}",
                                      name="gtw")
                    nc.tensor.matmul(ps[:], wo_sb[:, 0, et * P:(et + 1) * P],
                                     zt[j][:, 0, :], start=True, stop=False)
                nc.tensor.matmul(ps[:], woh_sb[:, 0, et * P:(et + 1) * P],
                                 zh[0][:], start=False, stop=False)
                nc.tensor.matmul(ps[:], woh_sb[:, 1, et * P:(et + 1) * P],
                                 zh[1][:], start=False, stop=True)
                ot = op_.tile([P, SB], BF16, tag="ott", name="ott")
                if et % 2 == 0:
                    nc.scalar.copy(ot[:], ps[:])
                else:
                    nc.vector.tensor_copy(ot[:], ps[:])
                if et < NDK - 1:
                    eng = nc.sync if et % 2 == 0 else nc.scalar
                    eng.dma_start(
                        outT[:][et * P:(et + 1) * P, j * SB:(j + 1) * SB], ot[:])
                else:
                    # split the final transfer across two queues to halve drain
                    nc.sync.dma_start(
                        outT[:][et * P:(et + 1) * P, j * SB:j * SB + 256],
                        ot[:, 0:256])
                    nc.scalar.dma_start(
                        outT[:][et * P:(et + 1) * P, j * SB + 256:(j + 1) * SB],
                        ot[:, 256:SB])

    nc.compile()
    return nc


def _host_inputs(inputs, Wq, bq, Wk, bk, Wv, bv, Wo, bo):
    """Build the 8 per-core input maps (bf16 weights/activations)."""
    import ml_dtypes
    bf16 = ml_dtypes.bfloat16
    scale = np.float32(1.0 / np.sqrt(HD))
    tri = np.ascontiguousarray(
        (np.arange(P)[:, None] <= np.arange(P)[None, :]).astype(bf16))
    xT_b = [np.ascontiguousarray(np.asarray(inputs[b], np.float32).T.astype(bf16))
            for b in range(B)]
    in_maps = []
    for c in range(NCORES):
        hg = c % 4
        hs = slice(4 * hg, 4 * hg + 4)
        WqT = np.asarray(Wq[hs], np.float32).transpose(2, 0, 1).reshape(D, 256) * scale
        WkT = np.asarray(Wk[hs], np.float32).transpose(2, 0, 1).reshape(D, 256)
        WvT = np.asarray(Wv[hs], np.float32).transpose(2, 0, 1).reshape(D, 256)
        wqkT = np.ascontiguousarray(
            np.concatenate([WqT, WkT], axis=1).astype(bf16))
        bq_c = np.asarray(bq[hs], np.float32).reshape(256) * scale
        bk_c = np.asarray(bk[hs], np.float32).reshape(256)
        bqk_c = np.stack([bq_c[0:128], bq_c[128:256], bk_c[0:128], bk_c[128:256]],
                         axis=1)
        woT = np.ascontiguousarray(
            np.asarray(Wo, np.float32)[:, 256 * hg:256 * (hg + 1)].T.astype(bf16))
        woh_c = np.ascontiguousarray(
            woT[128:256].reshape(2, 64, D).transpose(1, 0, 2).reshape(64, 2 * D))
        in_maps.append({
            "xT": xT_b[c // 4], "wqkT": wqkT,
            "wvT": np.ascontiguousarray(WvT.astype(bf16)), "woT": woT,
            "woh": woh_c,
            "bqk": np.ascontiguousarray(bqk_c), "tri": tri,
        })
    return in_maps


def _assemble(results, Wo, bv, bo):
    out = np.zeros((B, S, D), dtype=np.float32)
    for c in range(NCORES):
        out[c // 4] += results[c]["outT"].astype(np.float32).T
    bo_eff = (np.asarray(bo, np.float32)
              + np.asarray(Wo, np.float32) @ np.asarray(bv, np.float32).reshape(-1))
    out += bo_eff[None, None, :]
    return out


def kernel(inputs, Wq, bq, Wk, bk, Wv, bv, Wo, bo):
    from concourse.bass_utils import run_bass_kernel_spmd

    if "nc" not in _CACHE:
        _CACHE["nc"] = _build_nc()
    nc = _CACHE["nc"]
    in_maps = _host_inputs(inputs, Wq, bq, Wk, bk, Wv, bv, Wo, bo)
    res = run_bass_kernel_spmd(nc, in_maps, list(range(NCORES)))
    return _assemble(res.results, Wo, bv, bo)


# revision 65
# speedup vs baseline: 1.0466x; 1.0133x over previous
"""Causal multi-head attention (B=2, S=2048, D=1024, H=16) on 8 TRN2 NeuronCores.

Sharding: core c handles batch b=c//4 and the 4 heads [4*(c%4), 4*(c%4)+4).
Each core computes its heads' Q/K/V projections, causal attention, and a
column-shard of the output projection; the host sums the 4 partials per batch
and adds bo_eff = bo + Wo@bv (v-bias folded out of the device kernel).

v3 design (v2 + tail/startup/pipeline fixes; ~165us, was ~172us):
  - everything in bf16 on the PE (fp8 DoubleRow was tried and measured SLOWER
    per unit work on this toolchain: 268ns vs 213ns per 512-col matmul)
  - block-causal at 128-k granularity on the diagonal 512x512 region of each
    q-block; causal mask as a 0/1 multiply on probs (DVE) for the triangular
    128x128 sub-tile of each diagonal k-tile
  - AV stationary carries a 65th ones-column -> row 64 of wv psum is the
    softmax denominator
  - software-pipelined emission: scores(kt) -> exp(kt) -> AV(kt-1), with the
    next block's projections and the previous block's O-projection injected
    as PE filler between attention steps
  - v3 fixes: the last block's p=1 normalize keeps z halves at partitions
    0..63 (no SBUF->SBUF partition-shift DMA on the critical path; the tail
    O-proj contracts them via a 64-row copy of Wo), reciprocal broadcast via
    PE outer product, fillers dumped before the last normalize to keep the PE
    p-state warm, tail psums alternate psG/psW/psS banks for 4-deep
    pipelining, startup x/wqk DMAs split per dk-chunk over 3 queues, tail
    outT writes in 512-col (1KB-line) chunks with the final transfer split
    across two queues
"""

import numpy as np

B, S, D, H = 2, 2048, 1024, 16
HD = D // H  # 64
NCORES = 8
P = 128
SB = 512          # s/q block size
NSB = S // SB     # 4
NDK = D // P      # 8

_CACHE = {}


def _build_nc():
    import concourse.bacc as bacc
    import concourse.mybir as mybir
    import concourse.tile as tile

    F32 = mybir.dt.float32
    BF16 = mybir.dt.bfloat16
    EXP = mybir.ActivationFunctionType.Exp
    MULT = mybir.AluOpType.mult

    nc = bacc.Bacc(None)
    xT = nc.declare_dram_parameter("xT", [D, S], BF16, isOutput=False)
    wqk = nc.declare_dram_parameter("wqkT", [D, 512], BF16, isOutput=False)
    wv = nc.declare_dram_parameter("wvT", [D, 256], BF16, isOutput=False)
    wo = nc.declare_dram_parameter("woT", [256, D], BF16, isOutput=False)
    woh = nc.declare_dram_parameter("woh", [64, 2 * D], BF16, isOutput=False)
    bqk = nc.declare_dram_parameter("bqk", [P, 4], F32, isOutput=False)
    tri = nc.declare_dram_parameter("tri", [P, P], BF16, isOutput=False)
    outT = nc.declare_dram_parameter("outT", [D, S], BF16, isOutput=True)

    with tile.TileContext(nc) as tc:
        with (
            tc.tile_pool(name="w", bufs=1) as wp,
            tc.tile_pool(name="x", bufs=2) as xp,
            tc.tile_pool(name="qk", bufs=1) as qkp,
            tc.tile_pool(name="pb", bufs=4) as pbp,
            tc.tile_pool(name="sm", bufs=2) as smp,
            tc.tile_pool(name="o", bufs=4) as op_,
            tc.tile_pool(name="psS", bufs=2, space="PSUM") as psS,   # sc: 2x2 banks
            tc.tile_pool(name="psW", bufs=1, space="PSUM") as psW,   # wv0+wv1: 2 banks
            tc.tile_pool(name="psG", bufs=2, space="PSUM") as psG,   # proj/oproj: 2 banks
        ):
            # ---- weights / constants (DMA order = need order, 3 queues;
            # block-0 x and wqk split per dk-chunk so the first matmul only
            # waits on 256KB) ----
            xT_r = xT[:].rearrange("(dk p) s -> p dk s", p=P)
            wqk_r = wqk[:].rearrange("(dk p) m -> p dk m", p=P)
            wqk_t = [wp.tile([P, 512], BF16, name=f"wqk{k}") for k in range(8)]
            x0 = [xp.tile([P, SB], BF16, tag=f"x0k{k}", name=f"x0k{k}")
                  for k in range(8)]
            dq = (nc.sync, nc.scalar, nc.gpsimd)
            for k in range(8):
                dq[(2 * k) % 3].dma_start(wqk_t[k][:], wqk_r[:, k, :])
                dq[(2 * k + 1) % 3].dma_start(x0[k][:], xT_r[:, k, 0:SB])
            bqk_sb = wp.tile([P, 4], F32)
            nc.scalar.dma_start(bqk_sb[:], bqk[:])
            tri_sb = wp.tile([P, P], BF16)
            nc.gpsimd.dma_start(tri_sb[:], tri[:])
            # gate the non-critical weight loads behind the last critical x
            # chunk so they don't steal HBM bandwidth from the startup path
            gate = wp.tile([1, 8], BF16)
            nc.sync.dma_start(gate[:], x0[7][0:1, 0:8])
            wv_sb = wp.tile([P, NDK, 256], BF16)
            nc.sync.dma_start(wv_sb[:], wv[:].rearrange("(dk p) m -> p dk m", p=P))
            wo_sb = wp.tile([P, 2, D], BF16)
            woh_sb = wp.tile([HD, 2, D], BF16)
            ones_b = wp.tile([1, HD], BF16)
            nc.vector.memset(ones_b[:], 1.0)

            # ---- persistent activations (split per block: no false deps) ----
            qT = [[qkp.tile([P, SB], BF16, tag=f"qT{p}b{b}", name=f"qT{p}b{b}")
                   for b in range(NSB)] for p in range(2)]
            kT = [[qkp.tile([P, SB], BF16, tag=f"kT{p}b{b}", name=f"kT{p}b{b}")
                   for b in range(NSB)] for p in range(2)]
            vt = [qkp.tile([P, 4, 4, HD + 1], BF16, tag=f"v{b}", name=f"v{b}") for b in range(NSB)]
            for b in range(NSB):
                nc.vector.memset(vt[b][:, :, :, HD:HD + 1], 1.0)
            zt = [qkp.tile([P, 2, SB], BF16, tag=f"zT{b}", name=f"zT{b}") for b in range(NSB)]
            # last block p=1 z halves stay at partitions 0..63 (no shift DMA)
            zh = [qkp.tile([HD, SB], BF16, tag=f"zh{h}", name=f"zh{h}") for h in range(2)]

            x_tiles = [x0]

            def xap(blk, dk, c0=0, c1=SB):
                """x chunk dk of block blk as a [P, c1-c0] AP."""
                xs = x_tiles[blk]
                if len(xs) == 8:
                    return xs[dk][:, c0:c1]
                return xs[dk // 2][:, dk % 2, c0:c1]

            # ---- emission helpers ----
            def proj_chunks(blk):
                """Closures emitting block blk's Q/K/V projections."""
                out = []
                for t in range(2):          # 0 = q, 1 = k
                    for pair in range(2):
                        def qk_group(t=t, pair=pair, blk=blk):
                            ps = psG.tile([P, SB], F32, tag="g", name="g")
                            c0 = 256 * t + 128 * pair
                            for dk in range(NDK):
                                nc.tensor.matmul(
                                    ps[:], wqk_t[dk][:, c0:c0 + 128],
                                    xap(blk, dk),
                                    start=(dk == 0), stop=(dk == NDK - 1))
                            dst = (qT if t == 0 else kT)[pair][blk]
                            nc.vector.tensor_scalar_add(
                                dst[:], ps[:], bqk_sb[:, 2 * t + pair:2 * t + pair + 1])
                        out.append(qk_group)
                for half in range(2):        # two k-tiles of v per group
                    def v_group(half=half, blk=blk):
                        ps = psG.tile([P, 2, 256], F32, tag="g", name="gv")
                        for u in range(2):
                            c = 2 * half + u
                            for dk in range(NDK):
                                nc.tensor.matmul(
                                    ps[:, u, :],
                                    xap(blk, dk, c * P, (c + 1) * P),
                                    wv_sb[:, dk, :],
                                    start=(dk == 0), stop=(dk == NDK - 1))
                        nc.vector.tensor_copy(
                            vt[blk][:, 2 * half:2 * half + 2, :, 0:HD],
                            ps[:].rearrange("p two (h e) -> p two h e", h=4))
                    out.append(v_group)
                return out

            def oproj_chunks(j, alt=False):
                out = []
                for et in range(NDK):
                    def o_group(et=et, j=j, alt=alt):
                        if alt and et % 2 == 1:
                            # attention's sc banks are free once this emits
                            # (only released in the last-p dump)
                            pst = psS.tile([P, 2, SB], F32, tag="sc", name="gs")
                            ps = pst[:, 0, :]
                        else:
                            ps = psG.tile([P, SB], F32, tag="g", name="g")[:]
                        nc.tensor.matmul(ps, wo_sb[:, 0, et * P:(et + 1) * P],
                                         zt[j][:, 0, :], start=True, stop=False)
                        nc.tensor.matmul(ps, wo_sb[:, 1, et * P:(et + 1) * P],
                                         zt[j][:, 1, :], start=False, stop=True)
                        ot = op_.tile([P, SB], BF16, tag="ot", name="ot")
                        if alt and et >= 4:
                            nc.scalar.copy(ot[:], ps)
                        else:
                            nc.vector.tensor_copy(ot[:], ps)
                        nc.sync.dma_start(
                            outT[:][et * P:(et + 1) * P, j * SB:(j + 1) * SB], ot[:])
                    out.append(o_group)
                return out

            def x_prefetch(nblk):
                def go(nblk=nblk):
                    xn = [xp.tile([P, 2, SB], BF16, tag=f"xc{c}", name=f"x{nblk}c{c}")
                          for c in range(4)]
                    x_tiles.append(xn)
                    for c in range(4):
                        nc.sync.dma_start(
                            xn[c][:],
                            xT_r[:, 2 * c:2 * c + 2, nblk * SB:(nblk + 1) * SB])
                return go

            # ---- block 0 projections (nothing to interleave with yet) ----
            for ch in proj_chunks(0):
                ch()
            nc.sync.dma_start(wo_sb[:], wo[:].rearrange("(k p) m -> p k m", p=P))
            nc.sync.dma_start(woh_sb[:], woh[:].rearrange("p (k m) -> p k m", k=2))

            # ---- main loop: attention(j) + interleaved proj(j+1)/oproj(j-1) --
            for j in range(NSB):
                fillers = []
                if j + 1 < NSB:
                    fillers.append(x_prefetch(j + 1))
                    pj = proj_chunks(j + 1)
                else:
                    pj = []
                # defer O-projections so attn(2)/attn(3) have enough filler
                if j == 2:
                    oj = oproj_chunks(0)
                elif j == 3:
                    oj = oproj_chunks(1) + oproj_chunks(2, alt=True)
                else:
                    oj = []
                n = max(len(pj), len(oj))
                for i in range(n):
                    if i < len(pj):
                        fillers.append(pj[i])
                    if i < len(oj):
                        fillers.append(oj[i])

                total_slots = 2 * (4 * j + 4)
                slot = 0
                emitted = 0
                for p in range(2):
                    wv_t = [psW.tile([P, SB], F32, tag=f"wv{h}", name=f"wv{h}")
                            for h in range(2)]

                    def emit_av(item, p=p, j=j, wv_t=wv_t):
                        pr, koff, kti = item
                        first = (kti == 0)
                        last = (kti == 4 * j + 3)
                        for half in range(2):
                            nc.tensor.matmul(
                                wv_t[half][0:HD + 1, koff:SB],
                                vt[kti // 4][:, kti % 4, 2 * p + half, :],
                                pr[:, half, koff:SB],
                                start=first, stop=last, skip_group_check=True)

                    kts = list(range(4 * j)) + [("d", kk) for kk in range(4)]
                    prev = None
                    for i, kt in enumerate(kts):
                        if isinstance(kt, int):
                            koff = 0
                            kti = kt
                        else:
                            kk = kt[1]
                            koff = kk * P
                            kti = 4 * j + kk
                        sc = psS.tile([P, 2, SB], F32, tag="sc", name="sc")
                        for half in range(2):
                            base = HD * half
                            nc.tensor.matmul(
                                sc[:, half, koff:SB],
                                kT[p][kti // 4][base:base + HD,
                                                (kti % 4) * P:(kti % 4 + 1) * P],
                                qT[p][j][base:base + HD, koff:SB],
                                start=True, stop=True)
                        pr = pbp.tile([P, 2, SB], BF16, tag="pr", name="pr")
                        nc.scalar.activation(pr[:, :, koff:SB], sc[:, :, koff:SB], EXP)
                        if not isinstance(kt, int):
                            # 0/1 causal mask on the triangular 128x128 sub-tile
                            nc.vector.tensor_tensor(
                                pr[:, :, koff:koff + P], pr[:, :, koff:koff + P],
                                tri_sb[:, None, :].to_broadcast([P, 2, P]), MULT)
                        if prev is not None:
                            emit_av(prev)
                        prev = (pr, koff, kti)
                        slot += 1
                        # hold back fillers near each p's end so normalize's DVE
                        # ops aren't queued behind filler copies
                        if i < len(kts) - 2:
                            cap = len(fillers) - 8 if j == NSB - 1 else len(fillers)
                            want = min(cap,
                                       (slot * len(fillers) * 5)
                                       // (4 * max(1, total_slots - 4)))
                            while emitted < want:
                                fillers[emitted]()
                                emitted += 1
                    emit_av(prev)

                    # ---- normalize: denominators from psum row 64 ----
                    last_p = (j == NSB - 1 and p == 1)
                    if last_p:
                        # keep the PE warm through the normalize chain; the
                        # last 4 chunks (scalar copies) emit after normalize
                        while emitted < len(fillers) - 4:
                            fillers[emitted]()
                            emitted += 1
                    dns, wvss = [], []
                    for half in range(2):
                        # front-load the PSUM reads so the wv banks free up
                        # before the next p's first AV needs them
                        dn = smp.tile([1, SB], F32, tag=f"dn{half}",
                                      name=f"dn{half}")
                        wvs = smp.tile([HD, SB], F32, tag=f"wvs{half}",
                                       name=f"wvs{half}")
                        if last_p:
                            nc.scalar.copy(dn[:], wv_t[half][HD:HD + 1, :])
                            nc.scalar.copy(wvs[:], wv_t[half][0:HD, :])
                        else:
                            nc.vector.tensor_copy(dn[:], wv_t[half][HD:HD + 1, :])
                            nc.vector.tensor_copy(wvs[:], wv_t[half][0:HD, :])
                        dns.append(dn)
                        wvss.append(wvs)
                    for half in range(2):
                        dn, wvs = dns[half], wvss[half]
                        rr = smp.tile([1, SB], F32, tag="rr", name="rr")
                        nc.vector.reciprocal_approx_fast(rr[:], dn[:])
                        if last_p:
                            # short-latency path: PE outer-product broadcast,
                            # z halves stay at partitions 0..63
                            rrb = smp.tile([1, SB], BF16, tag="rrb", name="rrb")
                            nc.vector.tensor_copy(rrb[:], rr[:])
                            rbp = psG.tile([HD, SB], F32, tag="g", name="rbp")
                            nc.tensor.matmul(rbp[:], ones_b[:], rrb[:],
                                             start=True, stop=True)
                            nc.vector.tensor_tensor(
                                zh[half][:], wvs[:], rbp[:], MULT)
                        else:
                            rb = smp.tile([HD, SB], F32, tag="rb", name="rb")
                            nc.gpsimd.partition_broadcast(rb[:], rr[:])
                            if half == 0:
                                nc.vector.tensor_tensor(
                                    zt[j][0:HD, p, :], wvs[:], rb[:], MULT)
                            else:
                                zo = smp.tile([HD, SB], BF16, tag="zo", name="zo")
                                nc.vector.tensor_tensor(
                                    zo[:], wvs[:], rb[:], MULT)
                                nc.sync.dma_start(zt[j][HD:P, p, :], zo[:])

                while emitted < len(fillers):
                    fillers[emitted]()
                    emitted += 1

            # ---- tail: O-projection of the last block ----
            # zt[3][:,0,:] holds pair 0 (128-row contraction); pair 1 lives in
            # zh[0]/zh[1] at partitions 0..63, contracted via woh (64-row Wo).
            j = NSB - 1
            pss = {}
            for et in range(4):
                # pre-start the p0-half while the normalize chain runs;
                # zt[3][:,0,:] is ready well before zh[0]/zh[1]
                if et % 2 == 0:
                    ps = psG.tile([P, SB], F32, tag="g", name="gt")
                else:
                    ps = psW.tile([P, SB], F32, tag=f"wv{(et // 2) % 2}",
                                  name="gtw")
                pss[et] = ps
                nc.tensor.matmul(ps[:], wo_sb[:, 0, et * P:(et + 1) * P],
                                 zt[j][:, 0, :], start=True, stop=False)
            for et in range(NDK):
                if et < 4:
                    ps = pss[et]
                else:
                    if et % 2 == 0:
                        ps = psG.tile([P, SB], F32, tag="g", name="gt")
                    else:
                        ps = psW.tile([P, SB], F32, tag=f"wv{(et // 2) % 2}",
                                      name="gtw")
                    nc.tensor.matmul(ps[:], wo_sb[:, 0, et * P:(et + 1) * P],
                                     zt[j][:, 0, :], start=True, stop=False)
                nc.tensor.matmul(ps[:], woh_sb[:, 0, et * P:(et + 1) * P],
                                 zh[0][:], start=False, stop=False)
                nc.tensor.matmul(ps[:], woh_sb[:, 1, et * P:(et + 1) * P],
                                 zh[1][:], start=False, stop=True)
                ot = op_.tile([P, SB], BF16, tag="ott", name="ott")
                if et % 2 == 0:
                    nc.scalar.copy(ot[:], ps[:])
                else:
                    nc.vector.tensor_copy(ot[:], ps[:])
                if et < NDK - 1:
                    eng = nc.sync if et % 2 == 0 else nc.scalar
                    eng.dma_start(
                        outT[:][et * P:(et + 1) * P, j * SB:(j + 1) * SB], ot[:])
                else:
                    # split the final transfer across two queues to halve drain
                    nc.sync.dma_start(
                        outT[:][et * P:(et + 1) * P, j * SB:j * SB + 256],
                        ot[:, 0:256])
                    nc.scalar.dma_start(
                        outT[:][et * P:(et + 1) * P, j * SB + 256:(j + 1) * SB],
                        ot[:, 256:SB])

    nc.compile()
    return nc


def _host_inputs(inputs, Wq, bq, Wk, bk, Wv, bv, Wo, bo):
    """Build the 8 per-core input maps (bf16 weights/activations)."""
    import ml_dtypes
    bf16 = ml_dtypes.bfloat16
    scale = np.float32(1.0 / np.sqrt(HD))
    tri = np.ascontiguousarray(
        (np.arange(P)[:, None] <= np.arange(P)[None, :]).astype(bf16))
    xT_b = [np.ascontiguousarray(np.asarray(inputs[b], np.float32).T.astype(bf16))
            for b in range(B)]
    in_maps = []
    for c in range(NCORES):
        hg = c % 4
        hs = slice(4 * hg, 4 * hg + 4)
        WqT = np.asarray(Wq[hs], np.float32).transpose(2, 0, 1).reshape(D, 256) * scale
        WkT = np.asarray(Wk[hs], np.float32).transpose(2, 0, 1).reshape(D, 256)
        WvT = np.asarray(Wv[hs], np.float32).transpose(2, 0, 1).reshape(D, 256)
        wqkT = np.ascontiguousarray(
            np.concatenate([WqT, WkT], axis=1).astype(bf16))
        bq_c = np.asarray(bq[hs], np.float32).reshape(256) * scale
        bk_c = np.asarray(bk[hs], np.float32).reshape(256)
        bqk_c = np.stack([bq_c[0:128], bq_c[128:256], bk_c[0:128], bk_c[128:256]],
                         axis=1)
        woT = np.ascontiguousarray(
            np.asarray(Wo, np.float32)[:, 256 * hg:256 * (hg + 1)].T.astype(bf16))
        woh_c = np.ascontiguousarray(
            woT[128:256].reshape(2, 64, D).transpose(1, 0, 2).reshape(64, 2 * D))
        in_maps.append({
            "xT": xT_b[c // 4], "wqkT": wqkT,
            "wvT": np.ascontiguousarray(WvT.astype(bf16)), "woT": woT,
            "woh": woh_c,
            "bqk": np.ascontiguousarray(bqk_c), "tri": tri,
        })
    return in_maps


def _assemble(results, Wo, bv, bo):
    out = np.zeros((B, S, D), dtype=np.float32)
    for c in range(NCORES):
        out[c // 4] += results[c]["outT"].astype(np.float32).T
    bo_eff = (np.asarray(bo, np.float32)
              + np.asarray(Wo, np.float32) @ np.asarray(bv, np.float32).reshape(-1))
    out += bo_eff[None, None, :]
    return out


def kernel(inputs, Wq, bq, Wk, bk, Wv, bv, Wo, bo):
    from concourse.bass_utils import run_bass_kernel_spmd

    if "nc" not in _CACHE:
        _CACHE["nc"] = _build_nc()
    nc = _CACHE["nc"]
    in_maps = _host_inputs(inputs, Wq, bq, Wk, bk, Wv, bv, Wo, bo)
    res = run_bass_kernel_spmd(nc, in_maps, list(range(NCORES)))
    return _assemble(res.results, Wo, bv, bo)
